# revision 14
# baseline (speedup 1.0000x reference)
"""Multi-head graph attention (GAT) Trainium2 kernel, v3.

Row-sharded across 8 NeuronCores: core i owns queries [i*1024, (i+1)*1024).

Math (per head h, with Wh = h @ W_h, a = Wh@a1, b = Wh@a2, s = a_i + b_j):
    e[i,j]  = leakyrelu(s, 0.2)
    attn    = softmax_j(where(adj>0, e, -9e15))
    out_h   = elu(attn @ Wh)
    out     = concat_h(out_h) @ Wp.T + bp

On-chip factorization (exact): exp(lrelu(s)) = exp(0.2s) * max(exp(0.8s), 1).
The per-query factor exp(0.2 a_i) cancels in softmax, so the unnormalized
weight used on-chip is
    pm[j,i] = adjT[j,i] * max(exp(0.8 a_i + 0.8 b_j), 1)
with vb02_j = exp(0.2 b_j) folded into the value stationaries host-side.

v3: the per-block weight planes are built with 2 cheap ops/plane instead of
exp + 2 DVE passes, using exp(0.8s) = ea_i * eb_j with both exp factors
host-precomputed, and the identity max(x,1) fused into tensor_scalar's op1:
  D-recipe (DVE):  ee = ts(ea4_h * eb_j[per-part] max 1)   [4x mode]
                   pm = tt(ee * mask)                       [2x mode]
  A-recipe (ACT):  ee = ACT exp(abc_h + b08_j)              [1x]
                   pm = stt((ee max 1) * mask)              [2x, one pass]
  G-recipe:        ee on DVE ts, pm = tt on GPSIMD
Engine pattern per head is tunable via GAT_PAT (default "AADG").
adjm (pre-scaled masks, 32 MB/core DMA) is gone; only adjT bf16 is read.
"""

import os
from contextlib import ExitStack

import numpy as np
import ml_dtypes

import concourse.bacc as bacc
import concourse.bass as bass
import concourse.mybir as mybir
import concourse.tile as tile

F32 = mybir.dt.float32
BF16 = mybir.dt.bfloat16

ALU = mybir.AluOpType
AF = mybir.ActivationFunctionType

N = 8192          # nodes
IN_F = 256        # input features
H = 4             # heads
DH = 64           # head dim
NCORES = 8
QN = N // NCORES  # queries per core (1024)
KB = N // 128     # key blocks of 128 (64)
QH = 2            # 512-wide query halves

BF16_NP = ml_dtypes.bfloat16


def build_nc():
    nc = bacc.Bacc("TRN2", target_bir_lowering=False, debug=False)

    # host-precomputed tensors
    whv_d = nc.declare_dram_parameter("whv", [128, KB * H * (DH + 1)], BF16, False)
    adjt_d = nc.declare_dram_parameter("adjt", [N, QN], BF16, False)
    a08_d = nc.declare_dram_parameter("a08", [4, QN], F32, False)      # 0.8*a
    ea08_d = nc.declare_dram_parameter("ea08", [4, QN], BF16, False)   # exp(0.8*a)
    b08_d = nc.declare_dram_parameter("b08", [128, 4 * KB], F32, False)    # 0.8*b
    eb08_d = nc.declare_dram_parameter("eb08", [128, 4 * KB], F32, False)  # exp(0.8*b)
    wpt_d = nc.declare_dram_parameter("wpt", [IN_F, IN_F], F32, False)  # Wp.T
    bp_d = nc.declare_dram_parameter("bp", [IN_F], F32, False)
    out = nc.declare_dram_parameter("out", [QN, IN_F], F32, True)

    # plane builds: first ACTX planes on ACT (exp); of the rest, the last
    # GTS builds go to GPSIMD (Pool), others DVE — all ts(mult, max).
    # mask-apply: plain mult with a {0,1} bf16 mask; the last GAND planes'
    # mults go to GPSIMD over queries [0, GQ), everything else on DVE.
    ACTX = int(os.environ.get("GAT_ACTX", "2"))
    GTS = int(os.environ.get("GAT_GTS", "2"))
    GAND = int(os.environ.get("GAT_GAND", "0"))
    GQ = int(os.environ.get("GAT_GQ", str(QN)))

    with ExitStack() as ctx:
        tc = ctx.enter_context(tile.TileContext(nc))

        persist = ctx.enter_context(tc.tile_pool(name="persist", bufs=1))
        whv = persist.tile([128, KB, H, DH + 1], BF16)
        abc = persist.tile([128, 4, QN], F32)      # broadcast 0.8*a rows
        eap4 = persist.tile([128, 4, QN], BF16)    # broadcast exp(0.8a) rows
        b08 = persist.tile([128, 4, KB], F32)
        eb08 = persist.tile([128, 4, KB], F32)
        wpt_sb = persist.tile([128, 2, IN_F], F32)
        bpb = persist.tile([128, IN_F], F32)
        ones_b = persist.tile([1, 128], BF16)
        ones_f32 = persist.tile([1, 128], F32)
        ones_f = persist.tile([1, 64], F32)

        # main-loop pools pinned before setup so slots don't alias setup tiles
        MBUFS = int(os.environ.get("GAT_BUFS", "4"))
        mloop = ctx.enter_context(tc.tile_pool(name="mloop", bufs=MBUFS))
        for _b in range(MBUFS):
            _t = mloop.tile([128, QN], BF16, tag="mt")
            nc.vector.memset(_t[0:1, 0:2], 0.0)
            _t = mloop.tile([128, 4, QN], BF16, tag="ee4")
            nc.vector.memset(_t[0:1, 0, 0:2], 0.0)
            _t = mloop.tile([128, 4, QN], BF16, tag="pm4")
            nc.vector.memset(_t[0:1, 0, 0:2], 0.0)

        # ---------------- setup: DMAs + row broadcasts ----------------
        nc.vector.memset(ones_b, 1.0)
        nc.vector.memset(ones_f32, 1.0)
        nc.vector.memset(ones_f, 1.0)

        nc.scalar.dma_start(b08, b08_d[:, :].rearrange("p (j k) -> p j k", j=4))
        nc.scalar.dma_start(eb08, eb08_d[:, :].rearrange("p (j k) -> p j k", j=4))
        nc.scalar.dma_start(wpt_sb, wpt_d[:, :].rearrange("(c p) w -> p c w", p=128))
        bp_ap = bp_d[:]
        nc.gpsimd.dma_start(bpb, bass.AP(tensor=bp_ap.tensor, offset=bp_ap.offset,
                                         ap=[[0, 128]] + list(bp_ap.ap)))
        # whv streamed in key-block chunks so the first main matmuls are not
        # gated on the full 4.25 MB stationary load
        whv_r = whv_d[:, :].rearrange("p (k h d) -> p k h d", k=KB, h=H)
        for wc in range(8):
            ks = slice(wc * (KB // 8), (wc + 1) * (KB // 8))
            nc.scalar.dma_start(whv[:, ks, :, :], whv_r[:, ks, :, :])

        WARMUP = int(os.environ.get("GAT_WARMUP", "16"))
        with tc.tile_pool(name="setup", bufs=1) as setup, \
             tc.tile_pool(name="spsum", bufs=4, space="PSUM") as spsum:
            a08row = setup.tile([1, 4, QN], F32)
            ea08row = setup.tile([1, 4, QN], BF16)
            nc.sync.dma_start(a08row, a08_d[:, :].rearrange("(o j) q -> o j q", o=1))
            nc.sync.dma_start(ea08row, ea08_d[:, :].rearrange("(o j) q -> o j q", o=1))
            # broadcast rows across 128 partitions via ones-matmuls
            for j in range(4):
                for qh in range(QH):
                    qsl = slice(qh * 512, (qh + 1) * 512)
                    pa = spsum.tile([128, 512], F32, tag="bc_a")
                    nc.tensor.matmul(pa, ones_f32, a08row[:, j, qsl])
                    nc.vector.tensor_copy(abc[:, j, qsl], pa)
                    pe = spsum.tile([128, 512], F32, tag="bc_e")
                    nc.tensor.matmul(pe, ones_b, ea08row[:, j, qsl])
                    nc.scalar.copy(eap4[:, j, qsl], pe)
            # PE warm-up: back-to-back dummy matmuls to flip HAM to 8/8
            # before the real MMs
            for w in range(WARMUP):
                pw = spsum.tile([128, 512], F32, tag="bc_a")
                nc.tensor.matmul(pw[:, 0:256], wpt_sb[:, 0, 0:128], wpt_sb[:, 1, :])

        # ---------------- main loop ----------------
        mpsum_cm = tc.tile_pool(name="mpsum", bufs=1, space="PSUM")
        mpsum = mpsum_cm.__enter__()
        acc = mpsum.tile([DH + 1, H, QH, 512], F32)

        # software pipeline: all matmuls for block kb are emitted DELAY
        # iterations later, so the strict-FIFO PE queue never
        # head-of-line-blocks on a slow producer.
        DELAY = int(os.environ.get("GAT_DELAY", "2"))
        pend = []

        def finish_block(item):
            kb0, pm4_0 = item
            for hs in range(H):
                for qh in range(QH):
                    nc.tensor.matmul(acc[:, hs, qh, :], whv[:, kb0, hs, :],
                                     pm4_0[:, hs, qh * 512:(qh + 1) * 512],
                                     start=(kb0 == 0), stop=(kb0 == KB - 1))

        for kb in range(KB):
            mt = mloop.tile([128, QN], BF16, tag="mt")
            nc.sync.dma_start(mt, adjt_d[kb * 128:(kb + 1) * 128, :])
            ee4 = mloop.tile([128, 4, QN], BF16, tag="ee4")
            pm4 = mloop.tile([128, 4, QN], BF16, tag="pm4")

            # dep-free ts builds first so FIFOs never stall on ACT
            for h in range(ACTX, H):
                eng = nc.gpsimd if h >= H - GTS else nc.vector
                eng.tensor_scalar(ee4[:, h, :], eap4[:, h, :],
                                  eb08[:, h, kb:kb + 1], 1.0,
                                  op0=ALU.mult, op1=ALU.max)
            for h in range(ACTX):
                nc.scalar.activation(ee4[:, h, :], abc[:, h, :], AF.Exp,
                                     bias=b08[:, h, kb:kb + 1], scale=1.0)
            # GPSIMD mask-mults (issued early; wait on their build + DMA)
            for h in range(H - GAND, H):
                nc.gpsimd.tensor_tensor(pm4[:, h, 0:GQ], ee4[:, h, 0:GQ],
                                        mt[:, 0:GQ], op=ALU.mult)
            # clamp ACT planes: ee = max(ee, 1) flat across the ACT pair
            if ACTX:
                nc.vector.tensor_scalar(ee4[:, 0:ACTX, :], ee4[:, 0:ACTX, :],
                                        1.0, None, op0=ALU.max)
            # DVE mask-mults, flattened across the non-GPS planes
            nd = H - GAND
            if nd > 0:
                mtb = bass.AP(tensor=mt.tensor, offset=mt.offset,
                              ap=[list(mt.ap[0]), [0, nd], list(mt.ap[1])])
                nc.vector.tensor_tensor(pm4[:, 0:nd, :], ee4[:, 0:nd, :],
                                        mtb, op=ALU.mult)
            for h in range(H - GAND, H):
                if GQ < QN:
                    nc.vector.tensor_tensor(pm4[:, h, GQ:], ee4[:, h, GQ:],
                                            mt[:, GQ:], op=ALU.mult)

            pend.append((kb, pm4))
            if len(pend) > DELAY:
                finish_block(pend.pop(0))

        for item in pend:
            finish_block(item)

        # ---------------- tail: normalize, elu, out-proj ----------------
        tailp = ctx.enter_context(tc.tile_pool(name="tailp", bufs=1))
        denr = tailp.tile([1, H, QN], F32)
        gfin = tailp.tile([128, 2, QN], F32)
        graw = tailp.tile([128, 2, QN], F32)
        ACT_RECIP = int(os.environ.get("GAT_ACT_RECIP", "1"))
        for hs in range(H):
            for qh in range(QH):
                qsl = slice(qh * 512, (qh + 1) * 512)
                if ACT_RECIP:
                    # 1/den = square(1/sqrt(den)) on ACT (den > 0), keeping
                    # the iterative-divide off the DVE critical path
                    nc.scalar.activation(denr[:, hs, qsl], acc[DH:DH + 1, hs, qh, :],
                                         AF.Abs_reciprocal_sqrt)
                    nc.vector.tensor_mul(denr[:, hs, qsl], denr[:, hs, qsl],
                                         denr[:, hs, qsl])
                else:
                    nc.vector.reciprocal(denr[:, hs, qsl], acc[DH:DH + 1, hs, qh, :])
            nc.vector.tensor_copy(
                graw[(hs % 2) * 64:(hs % 2) * 64 + 64, hs // 2, :],
                acc[0:DH, hs, :, :].rearrange("p a b -> p (a b)"))
        mpsum_cm.__exit__(None, None, None)

        with tc.tile_pool(name="tpsum", bufs=2, space="PSUM") as tpsum, \
             tc.tile_pool(name="opsum", bufs=4, space="PSUM") as opsum, \
             tc.tile_pool(name="tailw", bufs=4) as tailw:
            # normalize: broadcast 1/den across partitions via ones-matmul
            for j in range(2):
                for qh in range(QH):
                    qsl = slice(qh * 512, (qh + 1) * 512)
                    rps = tpsum.tile([128, 512], F32, tag="r_ps")
                    nc.tensor.matmul(rps[0:64, :], ones_f, denr[:, 2 * j, qsl])
                    nc.tensor.matmul(rps[64:128, :], ones_f, denr[:, 2 * j + 1, qsl])
                    nc.vector.tensor_mul(gfin[:, j, qsl], graw[:, j, qsl], rps)

            # elu(x) = relu(x) + exp(min(x, 0)) - 1
            for qh in range(QH):
                for j in range(2):
                    qsl = slice(qh * 512, (qh + 1) * 512)
                    t = tailw.tile([128, 512], F32, tag="elu_t")
                    nc.vector.tensor_scalar(t, gfin[:, j, qsl], 0.0, None,
                                            op0=ALU.min)
                    e = tailw.tile([128, 512], F32, tag="elu_e")
                    nc.scalar.activation(e, t, AF.Exp)
                    em1 = tailw.tile([128, 512], F32, tag="elu_em1")
                    nc.vector.tensor_scalar(em1, e, -1.0, None, op0=ALU.add)
                    nc.vector.scalar_tensor_tensor(gfin[:, j, qsl], gfin[:, j, qsl],
                                                   0.0, em1, op0=ALU.max, op1=ALU.add)

            for qc in range(QN // 128):
                qsl = slice(qc * 128, (qc + 1) * 128)
                po = opsum.tile([128, IN_F], F32, tag="out_ps")
                nc.tensor.matmul(po, gfin[:, 0, qsl], wpt_sb[:, 0, :],
                                 start=True, stop=False)
                nc.tensor.matmul(po, gfin[:, 1, qsl], wpt_sb[:, 1, :],
                                 start=False, stop=True)
                fin = tailw.tile([128, IN_F], F32, tag="fin")
                nc.vector.scalar_tensor_tensor(fin, po, 0.0, bpb,
                                               op0=ALU.add, op1=ALU.add)
                nc.sync.dma_start(out[qsl, :], fin)

    nc.compile()
    return nc


_NC_CACHE = {}
LAST_RESULTS = None


def _get_nc():
    if "nc" not in _NC_CACHE:
        _NC_CACHE["nc"] = build_nc()
    return _NC_CACHE["nc"]


def kernel(h, adj, W, a1, a2, Wp, bp):
    from concourse.bass_utils import run_bass_kernel_spmd

    h = np.asarray(h, dtype=np.float32)
    adj = np.asarray(adj)
    W = np.asarray(W, dtype=np.float32)
    a1 = np.asarray(a1, dtype=np.float32)
    a2 = np.asarray(a2, dtype=np.float32)
    Wp = np.asarray(Wp, dtype=np.float32)
    bp = np.asarray(bp, dtype=np.float32)

    # ---- host precompute (O(N d^2): ~1% of kernel FLOPs) ----
    Wh = np.einsum("ni,hid->nhd", h, W).astype(np.float32)     # [N, H, DH]
    asc = np.einsum("nhd,hd->hn", Wh, a1)                      # [H, N]
    bsc = np.einsum("nhd,hd->hn", Wh, a2)                      # [H, N]
    vb02 = np.exp(0.2 * bsc)                                   # [H, N]
    # value stationaries [128, KB, H, DH+1]: [Wh * vb02 | vb02]
    whv_f = np.concatenate(
        [Wh * vb02.T[:, :, None], vb02.T[:, :, None]], axis=2)  # [N, H, DH+1]
    whv_np = np.ascontiguousarray(
        whv_f.reshape(KB, 128, H, DH + 1).transpose(1, 0, 2, 3)
        .reshape(128, KB * H * (DH + 1)).astype(BF16_NP))
    b08_np = np.ascontiguousarray(
        (0.8 * bsc).T.reshape(KB, 128, H).transpose(1, 2, 0)
        .reshape(128, H * KB).astype(np.float32))
    eb08_np = np.ascontiguousarray(
        np.exp((0.8 * bsc)).T.reshape(KB, 128, H).transpose(1, 2, 0)
        .reshape(128, H * KB).astype(np.float32))
    wpt = np.ascontiguousarray(Wp.T)

    nc = _get_nc()
    in_maps = []
    for c in range(NCORES):
        qsl = slice(c * QN, (c + 1) * QN)
        adjt_bits = adj[qsl, :].T.astype(BF16_NP)            # [N, QN] {0,1}
        in_maps.append({
            "whv": whv_np,
            "adjt": np.ascontiguousarray(adjt_bits),
            "a08": np.ascontiguousarray(0.8 * asc[:, qsl]).astype(np.float32),
            "ea08": np.ascontiguousarray(np.exp(0.8 * asc[:, qsl])).astype(BF16_NP),
            "b08": b08_np,
            "eb08": eb08_np,
            "wpt": wpt,
            "bp": bp,
        })

    res = run_bass_kernel_spmd(nc, in_maps, core_ids=list(range(NCORES)))
    global LAST_RESULTS
    LAST_RESULTS = res
    return np.concatenate([r["out"] for r in res.results], axis=0)


# revision 15
# speedup vs baseline: 6.6255x; 6.6255x over previous
"""Multi-head graph attention (GAT) Trainium2 kernel, v3.

Row-sharded across 8 NeuronCores: core i owns queries [i*1024, (i+1)*1024).

Math (per head h, with Wh = h @ W_h, a = Wh@a1, b = Wh@a2, s = a_i + b_j):
    e[i,j]  = leakyrelu(s, 0.2)
    attn    = softmax_j(where(adj>0, e, -9e15))
    out_h   = elu(attn @ Wh)
    out     = concat_h(out_h) @ Wp.T + bp

On-chip factorization (exact): exp(lrelu(s)) = exp(0.2s) * max(exp(0.8s), 1).
The per-query factor exp(0.2 a_i) cancels in softmax, so the unnormalized
weight used on-chip is
    pm[j,i] = adjT[j,i] * max(exp(0.8 a_i + 0.8 b_j), 1)
with vb02_j = exp(0.2 b_j) folded into the value stationaries host-side.

v3: the per-block weight planes are built with 2 cheap ops/plane instead of
exp + 2 DVE passes, using exp(0.8s) = ea_i * eb_j with both exp factors
host-precomputed, and the identity max(x,1) fused into tensor_scalar's op1:
  D-recipe (DVE):  ee = ts(ea4_h * eb_j[per-part] max 1)   [4x mode]
                   pm = tt(ee * mask)                       [2x mode]
  A-recipe (ACT):  ee = ACT exp(abc_h + b08_j)              [1x]
                   pm = stt((ee max 1) * mask)              [2x, one pass]
  G-recipe:        ee on DVE ts, pm = tt on GPSIMD
Engine pattern per head is tunable via GAT_PAT (default "AADG").
adjm (pre-scaled masks, 32 MB/core DMA) is gone; only adjT bf16 is read.
"""

import os
from contextlib import ExitStack

import numpy as np
import ml_dtypes

import concourse.bacc as bacc
import concourse.bass as bass
import concourse.mybir as mybir
import concourse.tile as tile

F32 = mybir.dt.float32
BF16 = mybir.dt.bfloat16

ALU = mybir.AluOpType
AF = mybir.ActivationFunctionType

N = 8192          # nodes
IN_F = 256        # input features
H = 4             # heads
DH = 64           # head dim
NCORES = 8
QN = N // NCORES  # queries per core (1024)
KB = N // 128     # key blocks of 128 (64)
QH = 2            # 512-wide query halves

BF16_NP = ml_dtypes.bfloat16


def build_nc():
    nc = bacc.Bacc("TRN2", target_bir_lowering=False, debug=False)

    # host-precomputed tensors
    whv_d = nc.declare_dram_parameter("whv", [128, KB * H * (DH + 1)], BF16, False)
    adjt_d = nc.declare_dram_parameter("adjt", [N, QN], BF16, False)
    a08_d = nc.declare_dram_parameter("a08", [4, QN], F32, False)      # 0.8*a
    ea08_d = nc.declare_dram_parameter("ea08", [4, QN], BF16, False)   # exp(0.8*a)
    b08_d = nc.declare_dram_parameter("b08", [128, 4 * KB], F32, False)    # 0.8*b
    eb08_d = nc.declare_dram_parameter("eb08", [128, 4 * KB], F32, False)  # exp(0.8*b)
    wpt_d = nc.declare_dram_parameter("wpt", [IN_F, IN_F], F32, False)  # Wp.T
    bp_d = nc.declare_dram_parameter("bp", [IN_F], F32, False)
    out = nc.declare_dram_parameter("out", [QN, IN_F], F32, True)

    # plane builds: first ACTX planes on ACT (exp); of the rest, the last
    # GTS builds go to GPSIMD (Pool), others DVE — all ts(mult, max).
    # mask-apply: plain mult with a {0,1} bf16 mask; the last GAND planes'
    # mults go to GPSIMD over queries [0, GQ), everything else on DVE.
    ACTX = int(os.environ.get("GAT_ACTX", "2"))
    GTS = int(os.environ.get("GAT_GTS", "0"))
    GAND = int(os.environ.get("GAT_GAND", "1"))
    GQ = int(os.environ.get("GAT_GQ", str(QN)))

    with ExitStack() as ctx:
        tc = ctx.enter_context(tile.TileContext(nc))

        persist = ctx.enter_context(tc.tile_pool(name="persist", bufs=1))
        whv = persist.tile([128, KB, H, DH + 1], BF16)
        abc = persist.tile([128, 4, QN], F32)      # broadcast 0.8*a rows
        eap4 = persist.tile([128, 4, QN], BF16)    # broadcast exp(0.8a) rows
        b08 = persist.tile([128, 4, KB], F32)
        eb08 = persist.tile([128, 4, KB], F32)
        wpt_sb = persist.tile([128, 2, IN_F], F32)
        bpb = persist.tile([128, IN_F], F32)
        ones_b = persist.tile([1, 128], BF16)
        ones_f32 = persist.tile([1, 128], F32)
        ones_f = persist.tile([1, 64], F32)

        # main-loop pools pinned before setup so slots don't alias setup tiles
        MBUFS = int(os.environ.get("GAT_BUFS", "4"))
        mloop = ctx.enter_context(tc.tile_pool(name="mloop", bufs=MBUFS))
        for _b in range(MBUFS):
            _t = mloop.tile([128, QN], BF16, tag="mt")
            nc.vector.memset(_t[0:1, 0:2], 0.0)
            _t = mloop.tile([128, 4, QN], BF16, tag="ee4")
            nc.vector.memset(_t[0:1, 0, 0:2], 0.0)
            _t = mloop.tile([128, 4, QN], BF16, tag="pm4")
            nc.vector.memset(_t[0:1, 0, 0:2], 0.0)

        # ---------------- setup: DMAs + row broadcasts ----------------
        nc.vector.memset(ones_b, 1.0)
        nc.vector.memset(ones_f32, 1.0)
        nc.vector.memset(ones_f, 1.0)

        nc.scalar.dma_start(b08, b08_d[:, :].rearrange("p (j k) -> p j k", j=4))
        nc.scalar.dma_start(eb08, eb08_d[:, :].rearrange("p (j k) -> p j k", j=4))
        nc.scalar.dma_start(wpt_sb, wpt_d[:, :].rearrange("(c p) w -> p c w", p=128))
        bp_ap = bp_d[:]
        nc.gpsimd.dma_start(bpb, bass.AP(tensor=bp_ap.tensor, offset=bp_ap.offset,
                                         ap=[[0, 128]] + list(bp_ap.ap)))
        # whv streamed in key-block chunks so the first main matmuls are not
        # gated on the full 4.25 MB stationary load
        whv_r = whv_d[:, :].rearrange("p (k h d) -> p k h d", k=KB, h=H)
        for wc in range(8):
            ks = slice(wc * (KB // 8), (wc + 1) * (KB // 8))
            nc.scalar.dma_start(whv[:, ks, :, :], whv_r[:, ks, :, :])

        WARMUP = int(os.environ.get("GAT_WARMUP", "16"))
        with tc.tile_pool(name="setup", bufs=1) as setup, \
             tc.tile_pool(name="spsum", bufs=4, space="PSUM") as spsum:
            a08row = setup.tile([1, 4, QN], F32)
            ea08row = setup.tile([1, 4, QN], BF16)
            nc.sync.dma_start(a08row, a08_d[:, :].rearrange("(o j) q -> o j q", o=1))
            nc.sync.dma_start(ea08row, ea08_d[:, :].rearrange("(o j) q -> o j q", o=1))
            # broadcast rows across 128 partitions via ones-matmuls
            for j in range(4):
                for qh in range(QH):
                    qsl = slice(qh * 512, (qh + 1) * 512)
                    pa = spsum.tile([128, 512], F32, tag="bc_a")
                    nc.tensor.matmul(pa, ones_f32, a08row[:, j, qsl])
                    nc.vector.tensor_copy(abc[:, j, qsl], pa)
                    pe = spsum.tile([128, 512], F32, tag="bc_e")
                    nc.tensor.matmul(pe, ones_b, ea08row[:, j, qsl])
                    nc.scalar.copy(eap4[:, j, qsl], pe)
            # PE warm-up: back-to-back dummy matmuls to flip HAM to 8/8
            # before the real MMs
            for w in range(WARMUP):
                pw = spsum.tile([128, 512], F32, tag="bc_a")
                nc.tensor.matmul(pw[:, 0:256], wpt_sb[:, 0, 0:128], wpt_sb[:, 1, :])

        # ---------------- main loop ----------------
        mpsum_cm = tc.tile_pool(name="mpsum", bufs=1, space="PSUM")
        mpsum = mpsum_cm.__enter__()
        acc = mpsum.tile([DH + 1, H, QH, 512], F32)

        # software pipeline: all matmuls for block kb are emitted DELAY
        # iterations later, so the strict-FIFO PE queue never
        # head-of-line-blocks on a slow producer.
        DELAY = int(os.environ.get("GAT_DELAY", "2"))
        pend = []

        def finish_block(item):
            kb0, pm4_0 = item
            for hs in range(H):
                for qh in range(QH):
                    nc.tensor.matmul(acc[:, hs, qh, :], whv[:, kb0, hs, :],
                                     pm4_0[:, hs, qh * 512:(qh + 1) * 512],
                                     start=(kb0 == 0), stop=(kb0 == KB - 1))

        for kb in range(KB):
            mt = mloop.tile([128, QN], BF16, tag="mt")
            nc.sync.dma_start(mt, adjt_d[kb * 128:(kb + 1) * 128, :])
            ee4 = mloop.tile([128, 4, QN], BF16, tag="ee4")
            pm4 = mloop.tile([128, 4, QN], BF16, tag="pm4")

            # dep-free ts builds first so FIFOs never stall on ACT
            for h in range(ACTX, H):
                eng = nc.gpsimd if h >= H - GTS else nc.vector
                eng.tensor_scalar(ee4[:, h, :], eap4[:, h, :],
                                  eb08[:, h, kb:kb + 1], 1.0,
                                  op0=ALU.mult, op1=ALU.max)
            for h in range(ACTX):
                nc.scalar.activation(ee4[:, h, :], abc[:, h, :], AF.Exp,
                                     bias=b08[:, h, kb:kb + 1], scale=1.0)
            # GPSIMD mask-mults (issued early; wait on their build + DMA)
            for h in range(H - GAND, H):
                nc.gpsimd.tensor_tensor(pm4[:, h, 0:GQ], ee4[:, h, 0:GQ],
                                        mt[:, 0:GQ], op=ALU.mult)
            # clamp ACT planes: ee = max(ee, 1) flat across the ACT pair
            if ACTX:
                nc.vector.tensor_scalar(ee4[:, 0:ACTX, :], ee4[:, 0:ACTX, :],
                                        1.0, None, op0=ALU.max)
            # DVE mask-mults, flattened across the non-GPS planes
            nd = H - GAND
            if nd > 0:
                mtb = bass.AP(tensor=mt.tensor, offset=mt.offset,
                              ap=[list(mt.ap[0]), [0, nd], list(mt.ap[1])])
                nc.vector.tensor_tensor(pm4[:, 0:nd, :], ee4[:, 0:nd, :],
                                        mtb, op=ALU.mult)
            for h in range(H - GAND, H):
                if GQ < QN:
                    nc.vector.tensor_tensor(pm4[:, h, GQ:], ee4[:, h, GQ:],
                                            mt[:, GQ:], op=ALU.mult)

            pend.append((kb, pm4))
            if len(pend) > DELAY:
                finish_block(pend.pop(0))

        for item in pend:
            finish_block(item)

        # ---------------- tail: normalize, elu, out-proj ----------------
        tailp = ctx.enter_context(tc.tile_pool(name="tailp", bufs=1))
        denr = tailp.tile([1, H, QN], F32)
        gfin = tailp.tile([128, 2, QN], F32)
        graw = tailp.tile([128, 2, QN], F32)
        ACT_RECIP = int(os.environ.get("GAT_ACT_RECIP", "1"))
        for hs in range(H):
            for qh in range(QH):
                qsl = slice(qh * 512, (qh + 1) * 512)
                if ACT_RECIP:
                    # 1/den = square(1/sqrt(den)) on ACT (den > 0), keeping
                    # the iterative-divide off the DVE critical path
                    nc.scalar.activation(denr[:, hs, qsl], acc[DH:DH + 1, hs, qh, :],
                                         AF.Abs_reciprocal_sqrt)
                    nc.vector.tensor_mul(denr[:, hs, qsl], denr[:, hs, qsl],
                                         denr[:, hs, qsl])
                else:
                    nc.vector.reciprocal(denr[:, hs, qsl], acc[DH:DH + 1, hs, qh, :])
            nc.vector.tensor_copy(
                graw[(hs % 2) * 64:(hs % 2) * 64 + 64, hs // 2, :],
                acc[0:DH, hs, :, :].rearrange("p a b -> p (a b)"))
        mpsum_cm.__exit__(None, None, None)

        with tc.tile_pool(name="tpsum", bufs=2, space="PSUM") as tpsum, \
             tc.tile_pool(name="opsum", bufs=4, space="PSUM") as opsum, \
             tc.tile_pool(name="tailw", bufs=4) as tailw:
            # normalize: broadcast 1/den across partitions via ones-matmul
            for j in range(2):
                for qh in range(QH):
                    qsl = slice(qh * 512, (qh + 1) * 512)
                    rps = tpsum.tile([128, 512], F32, tag="r_ps")
                    nc.tensor.matmul(rps[0:64, :], ones_f, denr[:, 2 * j, qsl])
                    nc.tensor.matmul(rps[64:128, :], ones_f, denr[:, 2 * j + 1, qsl])
                    nc.vector.tensor_mul(gfin[:, j, qsl], graw[:, j, qsl], rps)

            # elu(x) = relu(x) + exp(min(x, 0)) - 1
            for qh in range(QH):
                for j in range(2):
                    qsl = slice(qh * 512, (qh + 1) * 512)
                    t = tailw.tile([128, 512], F32, tag="elu_t")
                    nc.vector.tensor_scalar(t, gfin[:, j, qsl], 0.0, None,
                                            op0=ALU.min)
                    e = tailw.tile([128, 512], F32, tag="elu_e")
                    nc.scalar.activation(e, t, AF.Exp)
                    em1 = tailw.tile([128, 512], F32, tag="elu_em1")
                    nc.vector.tensor_scalar(em1, e, -1.0, None, op0=ALU.add)
                    nc.vector.scalar_tensor_tensor(gfin[:, j, qsl], gfin[:, j, qsl],
                                                   0.0, em1, op0=ALU.max, op1=ALU.add)

            for qc in range(QN // 128):
                qsl = slice(qc * 128, (qc + 1) * 128)
                po = opsum.tile([128, IN_F], F32, tag="out_ps")
                nc.tensor.matmul(po, gfin[:, 0, qsl], wpt_sb[:, 0, :],
                                 start=True, stop=False)
                nc.tensor.matmul(po, gfin[:, 1, qsl], wpt_sb[:, 1, :],
                                 start=False, stop=True)
                fin = tailw.tile([128, IN_F], F32, tag="fin")
                nc.vector.scalar_tensor_tensor(fin, po, 0.0, bpb,
                                               op0=ALU.add, op1=ALU.add)
                nc.sync.dma_start(out[qsl, :], fin)

    nc.compile()
    return nc


_NC_CACHE = {}
LAST_RESULTS = None


def _get_nc():
    if "nc" not in _NC_CACHE:
        _NC_CACHE["nc"] = build_nc()
    return _NC_CACHE["nc"]


def kernel(h, adj, W, a1, a2, Wp, bp):
    from concourse.bass_utils import run_bass_kernel_spmd

    h = np.asarray(h, dtype=np.float32)
    adj = np.asarray(adj)
    W = np.asarray(W, dtype=np.float32)
    a1 = np.asarray(a1, dtype=np.float32)
    a2 = np.asarray(a2, dtype=np.float32)
    Wp = np.asarray(Wp, dtype=np.float32)
    bp = np.asarray(bp, dtype=np.float32)

    # ---- host precompute (O(N d^2): ~1% of kernel FLOPs) ----
    Wh = np.einsum("ni,hid->nhd", h, W).astype(np.float32)     # [N, H, DH]
    asc = np.einsum("nhd,hd->hn", Wh, a1)                      # [H, N]
    bsc = np.einsum("nhd,hd->hn", Wh, a2)                      # [H, N]
    vb02 = np.exp(0.2 * bsc)                                   # [H, N]
    # value stationaries [128, KB, H, DH+1]: [Wh * vb02 | vb02]
    whv_f = np.concatenate(
        [Wh * vb02.T[:, :, None], vb02.T[:, :, None]], axis=2)  # [N, H, DH+1]
    whv_np = np.ascontiguousarray(
        whv_f.reshape(KB, 128, H, DH + 1).transpose(1, 0, 2, 3)
        .reshape(128, KB * H * (DH + 1)).astype(BF16_NP))
    b08_np = np.ascontiguousarray(
        (0.8 * bsc).T.reshape(KB, 128, H).transpose(1, 2, 0)
        .reshape(128, H * KB).astype(np.float32))
    eb08_np = np.ascontiguousarray(
        np.exp((0.8 * bsc)).T.reshape(KB, 128, H).transpose(1, 2, 0)
        .reshape(128, H * KB).astype(np.float32))
    wpt = np.ascontiguousarray(Wp.T)

    nc = _get_nc()
    in_maps = []
    for c in range(NCORES):
        qsl = slice(c * QN, (c + 1) * QN)
        adjt_bits = adj[qsl, :].T.astype(BF16_NP)            # [N, QN] {0,1}
        in_maps.append({
            "whv": whv_np,
            "adjt": np.ascontiguousarray(adjt_bits),
            "a08": np.ascontiguousarray(0.8 * asc[:, qsl]).astype(np.float32),
            "ea08": np.ascontiguousarray(np.exp(0.8 * asc[:, qsl])).astype(BF16_NP),
            "b08": b08_np,
            "eb08": eb08_np,
            "wpt": wpt,
            "bp": bp,
        })

    res = run_bass_kernel_spmd(nc, in_maps, core_ids=list(range(NCORES)))
    global LAST_RESULTS
    LAST_RESULTS = res
    return np.concatenate([r["out"] for r in res.results], axis=0)


# revision 16
# speedup vs baseline: 8.7864x; 1.3261x over previous
"""Multi-head graph attention (GAT) Trainium2 kernel, v3.

Row-sharded across 8 NeuronCores: core i owns queries [i*1024, (i+1)*1024).

Math (per head h, with Wh = h @ W_h, a = Wh@a1, b = Wh@a2, s = a_i + b_j):
    e[i,j]  = leakyrelu(s, 0.2)
    attn    = softmax_j(where(adj>0, e, -9e15))
    out_h   = elu(attn @ Wh)
    out     = concat_h(out_h) @ Wp.T + bp

On-chip factorization (exact): exp(lrelu(s)) = exp(0.2s) * max(exp(0.8s), 1).
The per-query factor exp(0.2 a_i) cancels in softmax, so the unnormalized
weight used on-chip is
    pm[j,i] = adjT[j,i] * max(exp(0.8 a_i + 0.8 b_j), 1)
with vb02_j = exp(0.2 b_j) folded into the value stationaries host-side.

v3: the per-block weight planes are built with 2 cheap ops/plane instead of
exp + 2 DVE passes, using exp(0.8s) = ea_i * eb_j with both exp factors
host-precomputed, and the identity max(x,1) fused into tensor_scalar's op1:
  D-recipe (DVE):  ee = ts(ea4_h * eb_j[per-part] max 1)   [4x mode]
                   pm = tt(ee * mask)                       [2x mode]
  A-recipe (ACT):  ee = ACT exp(abc_h + b08_j)              [1x]
                   pm = stt((ee max 1) * mask)              [2x, one pass]
  G-recipe:        ee on DVE ts, pm = tt on GPSIMD
Engine pattern per head is tunable via GAT_PAT (default "AADG").
adjm (pre-scaled masks, 32 MB/core DMA) is gone; only adjT bf16 is read.
"""

import os
from contextlib import ExitStack

import numpy as np
import ml_dtypes

import concourse.bacc as bacc
import concourse.bass as bass
import concourse.mybir as mybir
import concourse.tile as tile

F32 = mybir.dt.float32
BF16 = mybir.dt.bfloat16

ALU = mybir.AluOpType
AF = mybir.ActivationFunctionType

N = 8192          # nodes
IN_F = 256        # input features
H = 4             # heads
DH = 64           # head dim
NCORES = 8
QN = N // NCORES  # queries per core (1024)
KB = N // 128     # key blocks of 128 (64)
QH = 2            # 512-wide query halves

BF16_NP = ml_dtypes.bfloat16


def build_nc():
    nc = bacc.Bacc("TRN2", target_bir_lowering=False, debug=False)

    # host-precomputed tensors
    whv_d = nc.declare_dram_parameter("whv", [128, KB * H * (DH + 1)], BF16, False)
    adjt_d = nc.declare_dram_parameter("adjt", [N, QN], BF16, False)
    a08_d = nc.declare_dram_parameter("a08", [4, QN], F32, False)      # 0.8*a
    ea08_d = nc.declare_dram_parameter("ea08", [4, QN], BF16, False)   # exp(0.8*a)
    b08_d = nc.declare_dram_parameter("b08", [128, 4 * KB], F32, False)    # 0.8*b
    eb08_d = nc.declare_dram_parameter("eb08", [128, 4 * KB], F32, False)  # exp(0.8*b)
    wpt_d = nc.declare_dram_parameter("wpt", [IN_F, IN_F], F32, False)  # Wp.T
    bp_d = nc.declare_dram_parameter("bp", [IN_F], F32, False)
    out = nc.declare_dram_parameter("out", [QN, IN_F], F32, True)

    # plane builds: first ACTX planes on ACT (exp); of the rest, the last
    # GTS builds go to GPSIMD (Pool), others DVE — all ts(mult, max).
    # mask-apply: plain mult with a {0,1} bf16 mask; the last GAND planes'
    # mults go to GPSIMD over queries [0, GQ), everything else on DVE.
    ACTX = int(os.environ.get("GAT_ACTX", "3"))
    GTS = int(os.environ.get("GAT_GTS", "0"))
    GAND = int(os.environ.get("GAT_GAND", "0"))
    GQ = int(os.environ.get("GAT_GQ", str(QN)))

    with ExitStack() as ctx:
        tc = ctx.enter_context(tile.TileContext(nc))

        persist = ctx.enter_context(tc.tile_pool(name="persist", bufs=1))
        whv = persist.tile([128, KB, H, DH + 1], BF16)
        abc = persist.tile([128, 4, QN], F32)      # broadcast 0.8*a rows
        eap4 = persist.tile([128, 4, QN], BF16)    # broadcast exp(0.8a) rows
        b08 = persist.tile([128, 4, KB], F32)
        eb08 = persist.tile([128, 4, KB], F32)
        wpt_sb = persist.tile([128, 2, IN_F], F32)
        bpb = persist.tile([128, IN_F], F32)
        ones_b = persist.tile([1, 128], BF16)
        ones_f32 = persist.tile([1, 128], F32)
        ones_f = persist.tile([1, 64], F32)

        # main-loop pools pinned before setup so slots don't alias setup tiles
        MBUFS = int(os.environ.get("GAT_BUFS", "4"))
        mloop = ctx.enter_context(tc.tile_pool(name="mloop", bufs=MBUFS))
        for _b in range(MBUFS):
            _t = mloop.tile([128, QN], BF16, tag="mt")
            nc.vector.memset(_t[0:1, 0:2], 0.0)
            _t = mloop.tile([128, 4, QN], BF16, tag="ee4")
            nc.vector.memset(_t[0:1, 0, 0:2], 0.0)
            _t = mloop.tile([128, 4, QN], BF16, tag="pm4")
            nc.vector.memset(_t[0:1, 0, 0:2], 0.0)

        # ---------------- setup: DMAs + row broadcasts ----------------
        nc.vector.memset(ones_b, 1.0)
        nc.vector.memset(ones_f32, 1.0)
        nc.vector.memset(ones_f, 1.0)

        nc.scalar.dma_start(b08, b08_d[:, :].rearrange("p (j k) -> p j k", j=4))
        nc.scalar.dma_start(eb08, eb08_d[:, :].rearrange("p (j k) -> p j k", j=4))
        nc.scalar.dma_start(wpt_sb, wpt_d[:, :].rearrange("(c p) w -> p c w", p=128))
        bp_ap = bp_d[:]
        nc.gpsimd.dma_start(bpb, bass.AP(tensor=bp_ap.tensor, offset=bp_ap.offset,
                                         ap=[[0, 128]] + list(bp_ap.ap)))
        # whv streamed in key-block chunks so the first main matmuls are not
        # gated on the full 4.25 MB stationary load
        whv_r = whv_d[:, :].rearrange("p (k h d) -> p k h d", k=KB, h=H)
        for wc in range(8):
            ks = slice(wc * (KB // 8), (wc + 1) * (KB // 8))
            nc.scalar.dma_start(whv[:, ks, :, :], whv_r[:, ks, :, :])

        WARMUP = int(os.environ.get("GAT_WARMUP", "16"))
        with tc.tile_pool(name="setup", bufs=1) as setup, \
             tc.tile_pool(name="spsum", bufs=4, space="PSUM") as spsum:
            a08row = setup.tile([1, 4, QN], F32)
            ea08row = setup.tile([1, 4, QN], BF16)
            nc.sync.dma_start(a08row, a08_d[:, :].rearrange("(o j) q -> o j q", o=1))
            nc.sync.dma_start(ea08row, ea08_d[:, :].rearrange("(o j) q -> o j q", o=1))
            # broadcast rows across 128 partitions via ones-matmuls
            for j in range(4):
                for qh in range(QH):
                    qsl = slice(qh * 512, (qh + 1) * 512)
                    pa = spsum.tile([128, 512], F32, tag="bc_a")
                    nc.tensor.matmul(pa, ones_f32, a08row[:, j, qsl])
                    nc.vector.tensor_copy(abc[:, j, qsl], pa)
                    pe = spsum.tile([128, 512], F32, tag="bc_e")
                    nc.tensor.matmul(pe, ones_b, ea08row[:, j, qsl])
                    nc.scalar.copy(eap4[:, j, qsl], pe)
            # PE warm-up: back-to-back dummy matmuls to flip HAM to 8/8
            # before the real MMs
            for w in range(WARMUP):
                pw = spsum.tile([128, 512], F32, tag="bc_a")
                nc.tensor.matmul(pw[:, 0:256], wpt_sb[:, 0, 0:128], wpt_sb[:, 1, :])

        # ---------------- main loop ----------------
        mpsum_cm = tc.tile_pool(name="mpsum", bufs=1, space="PSUM")
        mpsum = mpsum_cm.__enter__()
        acc = mpsum.tile([DH + 1, H, QH, 512], F32)

        # software pipeline: all matmuls for block kb are emitted DELAY
        # iterations later, so the strict-FIFO PE queue never
        # head-of-line-blocks on a slow producer.
        DELAY = int(os.environ.get("GAT_DELAY", "2"))
        pend = []

        def finish_block(item):
            kb0, pm4_0 = item
            for hs in range(H):
                for qh in range(QH):
                    nc.tensor.matmul(acc[:, hs, qh, :], whv[:, kb0, hs, :],
                                     pm4_0[:, hs, qh * 512:(qh + 1) * 512],
                                     start=(kb0 == 0), stop=(kb0 == KB - 1))

        for kb in range(KB):
            mt = mloop.tile([128, QN], BF16, tag="mt")
            nc.sync.dma_start(mt, adjt_d[kb * 128:(kb + 1) * 128, :])
            ee4 = mloop.tile([128, 4, QN], BF16, tag="ee4")
            pm4 = mloop.tile([128, 4, QN], BF16, tag="pm4")

            # dep-free ts builds first so FIFOs never stall on ACT
            for h in range(ACTX, H):
                eng = nc.gpsimd if h >= H - GTS else nc.vector
                eng.tensor_scalar(ee4[:, h, :], eap4[:, h, :],
                                  eb08[:, h, kb:kb + 1], 1.0,
                                  op0=ALU.mult, op1=ALU.max)
            for h in range(ACTX):
                nc.scalar.activation(ee4[:, h, :], abc[:, h, :], AF.Exp,
                                     bias=b08[:, h, kb:kb + 1], scale=1.0)
            # GPSIMD mask-mults (issued early; wait on their build + DMA)
            for h in range(H - GAND, H):
                nc.gpsimd.tensor_tensor(pm4[:, h, 0:GQ], ee4[:, h, 0:GQ],
                                        mt[:, 0:GQ], op=ALU.mult)
            # clamp ACT planes: ee = max(ee, 1) flat across the ACT pair
            if ACTX:
                nc.vector.tensor_scalar(ee4[:, 0:ACTX, :], ee4[:, 0:ACTX, :],
                                        1.0, None, op0=ALU.max)
            # DVE mask-mults, flattened across the non-GPS planes
            nd = H - GAND
            if nd > 0:
                mtb = bass.AP(tensor=mt.tensor, offset=mt.offset,
                              ap=[list(mt.ap[0]), [0, nd], list(mt.ap[1])])
                nc.vector.tensor_tensor(pm4[:, 0:nd, :], ee4[:, 0:nd, :],
                                        mtb, op=ALU.mult)
            for h in range(H - GAND, H):
                if GQ < QN:
                    nc.vector.tensor_tensor(pm4[:, h, GQ:], ee4[:, h, GQ:],
                                            mt[:, GQ:], op=ALU.mult)

            pend.append((kb, pm4))
            if len(pend) > DELAY:
                finish_block(pend.pop(0))

        for item in pend:
            finish_block(item)

        # ---------------- tail: normalize, elu, out-proj ----------------
        tailp = ctx.enter_context(tc.tile_pool(name="tailp", bufs=1))
        denr = tailp.tile([1, H, QN], F32)
        gfin = tailp.tile([128, 2, QN], F32)
        graw = tailp.tile([128, 2, QN], F32)
        ACT_RECIP = int(os.environ.get("GAT_ACT_RECIP", "1"))
        for hs in range(H):
            for qh in range(QH):
                qsl = slice(qh * 512, (qh + 1) * 512)
                if ACT_RECIP:
                    # 1/den = square(1/sqrt(den)) on ACT (den > 0), keeping
                    # the iterative-divide off the DVE critical path
                    nc.scalar.activation(denr[:, hs, qsl], acc[DH:DH + 1, hs, qh, :],
                                         AF.Abs_reciprocal_sqrt)
                    nc.vector.tensor_mul(denr[:, hs, qsl], denr[:, hs, qsl],
                                         denr[:, hs, qsl])
                else:
                    nc.vector.reciprocal(denr[:, hs, qsl], acc[DH:DH + 1, hs, qh, :])
            nc.scalar.copy(
                graw[(hs % 2) * 64:(hs % 2) * 64 + 64, hs // 2, :],
                acc[0:DH, hs, :, :].rearrange("p a b -> p (a b)"))
        mpsum_cm.__exit__(None, None, None)

        with tc.tile_pool(name="tpsum", bufs=2, space="PSUM") as tpsum, \
             tc.tile_pool(name="opsum", bufs=4, space="PSUM") as opsum, \
             tc.tile_pool(name="tailw", bufs=4) as tailw:
            # normalize: broadcast 1/den across partitions via ones-matmul
            for j in range(2):
                for qh in range(QH):
                    qsl = slice(qh * 512, (qh + 1) * 512)
                    rps = tpsum.tile([128, 512], F32, tag="r_ps")
                    nc.tensor.matmul(rps[0:64, :], ones_f, denr[:, 2 * j, qsl])
                    nc.tensor.matmul(rps[64:128, :], ones_f, denr[:, 2 * j + 1, qsl])
                    nc.vector.tensor_mul(gfin[:, j, qsl], graw[:, j, qsl], rps)

            # elu(x) = relu(x) + exp(min(x, 0)) - 1
            for qh in range(QH):
                for j in range(2):
                    qsl = slice(qh * 512, (qh + 1) * 512)
                    t = tailw.tile([128, 512], F32, tag="elu_t")
                    nc.vector.tensor_scalar(t, gfin[:, j, qsl], 0.0, None,
                                            op0=ALU.min)
                    e = tailw.tile([128, 512], F32, tag="elu_e")
                    nc.scalar.activation(e, t, AF.Exp)
                    em1 = tailw.tile([128, 512], F32, tag="elu_em1")
                    nc.vector.tensor_scalar(em1, e, -1.0, None, op0=ALU.add)
                    nc.vector.scalar_tensor_tensor(gfin[:, j, qsl], gfin[:, j, qsl],
                                                   0.0, em1, op0=ALU.max, op1=ALU.add)

            for qc in range(QN // 128):
                qsl = slice(qc * 128, (qc + 1) * 128)
                po = opsum.tile([128, IN_F], F32, tag="out_ps")
                nc.tensor.matmul(po, gfin[:, 0, qsl], wpt_sb[:, 0, :],
                                 start=True, stop=False)
                nc.tensor.matmul(po, gfin[:, 1, qsl], wpt_sb[:, 1, :],
                                 start=False, stop=True)
                fin = tailw.tile([128, IN_F], F32, tag="fin")
                nc.vector.scalar_tensor_tensor(fin, po, 0.0, bpb,
                                               op0=ALU.add, op1=ALU.add)
                nc.sync.dma_start(out[qsl, :], fin)

    nc.compile()
    return nc


_NC_CACHE = {}
LAST_RESULTS = None


def _get_nc():
    if "nc" not in _NC_CACHE:
        _NC_CACHE["nc"] = build_nc()
    return _NC_CACHE["nc"]


def kernel(h, adj, W, a1, a2, Wp, bp):
    from concourse.bass_utils import run_bass_kernel_spmd

    h = np.asarray(h, dtype=np.float32)
    adj = np.asarray(adj)
    W = np.asarray(W, dtype=np.float32)
    a1 = np.asarray(a1, dtype=np.float32)
    a2 = np.asarray(a2, dtype=np.float32)
    Wp = np.asarray(Wp, dtype=np.float32)
    bp = np.asarray(bp, dtype=np.float32)

    # ---- host precompute (O(N d^2): ~1% of kernel FLOPs) ----
    Wh = np.einsum("ni,hid->nhd", h, W).astype(np.float32)     # [N, H, DH]
    asc = np.einsum("nhd,hd->hn", Wh, a1)                      # [H, N]
    bsc = np.einsum("nhd,hd->hn", Wh, a2)                      # [H, N]
    vb02 = np.exp(0.2 * bsc)                                   # [H, N]
    # value stationaries [128, KB, H, DH+1]: [Wh * vb02 | vb02]
    whv_f = np.concatenate(
        [Wh * vb02.T[:, :, None], vb02.T[:, :, None]], axis=2)  # [N, H, DH+1]
    whv_np = np.ascontiguousarray(
        whv_f.reshape(KB, 128, H, DH + 1).transpose(1, 0, 2, 3)
        .reshape(128, KB * H * (DH + 1)).astype(BF16_NP))
    b08_np = np.ascontiguousarray(
        (0.8 * bsc).T.reshape(KB, 128, H).transpose(1, 2, 0)
        .reshape(128, H * KB).astype(np.float32))
    eb08_np = np.ascontiguousarray(
        np.exp((0.8 * bsc)).T.reshape(KB, 128, H).transpose(1, 2, 0)
        .reshape(128, H * KB).astype(np.float32))
    wpt = np.ascontiguousarray(Wp.T)

    nc = _get_nc()
    in_maps = []
    for c in range(NCORES):
        qsl = slice(c * QN, (c + 1) * QN)
        adjt_bits = adj[qsl, :].T.astype(BF16_NP)            # [N, QN] {0,1}
        in_maps.append({
            "whv": whv_np,
            "adjt": np.ascontiguousarray(adjt_bits),
            "a08": np.ascontiguousarray(0.8 * asc[:, qsl]).astype(np.float32),
            "ea08": np.ascontiguousarray(np.exp(0.8 * asc[:, qsl])).astype(BF16_NP),
            "b08": b08_np,
            "eb08": eb08_np,
            "wpt": wpt,
            "bp": bp,
        })

    res = run_bass_kernel_spmd(nc, in_maps, core_ids=list(range(NCORES)))
    global LAST_RESULTS
    LAST_RESULTS = res
    return np.concatenate([r["out"] for r in res.results], axis=0)
